# revision 15
# baseline (speedup 1.0000x reference)
"""Deformable sampling module (DCN-style bilinear gather + mask-weighted
tap accumulation) for Trainium2, 8 NeuronCores, data-parallel over batch.

Shapes (hardcoded): input [8, 256, 64, 64], offset [8, 72, 64, 64],
mask [8, 36, 64, 64] -> output [8, 256, 64, 64].
G=4 deformable groups, K=9 taps, Cg=64 channels/group.

v3: one ap_gather index per (tap, position) fetches the whole 2x2
bilinear patch (d=4) from an interleaved image
X4[c, 4q+e] = X[c, q + {0,1,64,65}[e]].  Patch base (yb, xb) =
clamp(floor(py), 0, 62) x clamp(floor(px), 0, 62); weights are tents
evaluated at the patch's integer positions, times mask.  Exact: every
integer grid point with nonzero tent weight lies inside the clamped
patch, and out-of-image points get tent weight 0.
"""
import sys
import numpy as np

sys.path.insert(0, "/opt/trn_rl_repo")

import concourse.bacc as bacc
import concourse.tile as tile
import concourse.mybir as mybir
from concourse import library_config
from concourse.vector_clock import ScopedClock
from concourse.bass_utils import run_bass_kernel_spmd

F32 = mybir.dt.float32
I16 = mybir.dt.int16

B, C, H, W = 8, 256, 64, 64
G, K, Cg = 4, 9, 64
HW = H * W
KY = np.arange(3).repeat(3)
KX = np.tile(np.arange(3), 3)
NCHUNK = 4
CH = HW // NCHUNK          # 1024 positions per chunk
S_CH = CH // 16            # 64 wrapped slots per chunk
KCOL = K * S_CH            # 576 idx columns per chunk (k-major)
MAGIC = float(3 << 22)     # floor trick constant (ulp=1 zone)
E4 = [(0, 0), (0, 1), (1, 0), (1, 1)]


def _patch_tile_drain():
    """walrus rejects >1 sync wait on the tile-exit Drain; spill extras
    onto preceding sync-engine nops."""
    if getattr(tile.TileContext, "_drain_patched", False):
        return

    def _drain_and_barrier(self, tick_clock, wait_clock):
        nc = self.nc
        drain_inst = nc.sync.drain()
        wait_clock.add_sem_waits(
            drain_inst.ins, ScopedClock({None: tick_clock.global_clock})
        )
        si = drain_inst.ins.sync_info
        if si is not None and len(si.on_wait) > 1:
            ow = list(si.on_wait)
            si.on_wait = ow[:1]
            for i in range(1, len(ow)):
                nop = nc.sync.nop(nofuse=True, hint="drain_wait_spill")
                nop.ins.sync_info = mybir.SyncInfo(
                    on_wait=[ow[i]], on_update=[]
                )
        nc.all_engine_barrier()
        assert self.sems is not None
        popped = nc._tile_sem_poison_stack.pop()
        assert popped is self._sem_poison
        nc.clear_and_free_semaphores(list(self.sems.allocated().values()))
        nc.all_engine_barrier()

    tile.TileContext._drain_and_barrier = _drain_and_barrier
    tile.TileContext._drain_patched = True


def _floor(nc, pool, out, src, shape, tagp):
    """out = floor(src), via fused round-magic + compare fixup."""
    r = pool.tile(shape, F32, tag=f"{tagp}_r")
    c = pool.tile(shape, F32, tag=f"{tagp}_c")
    nc.vector.tensor_scalar(
        out=r[:], in0=src, scalar1=MAGIC, scalar2=MAGIC,
        op0=mybir.AluOpType.add, op1=mybir.AluOpType.subtract,
    )
    nc.vector.tensor_tensor(c[:], r[:], src, mybir.AluOpType.is_gt)
    nc.vector.tensor_tensor(out, r[:], c[:], mybir.AluOpType.subtract)


def _clamp(nc, out, src, lo, hi):
    nc.vector.tensor_scalar(
        out=out, in0=src, scalar1=lo, scalar2=hi,
        op0=mybir.AluOpType.max, op1=mybir.AluOpType.min,
    )


def _tent(nc, pool, out, u, shape, tagp):
    """out = relu(1 - |u|)."""
    a = pool.tile(shape, F32, tag=f"{tagp}_a")
    nc.vector.tensor_scalar_mul(a[:], u, -1.0)
    nc.vector.tensor_tensor(a[:], a[:], u, mybir.AluOpType.max)
    nc.vector.tensor_scalar(
        out=a[:], in0=a[:], scalar1=-1.0, scalar2=1.0,
        op0=mybir.AluOpType.mult, op1=mybir.AluOpType.add,
    )
    nc.vector.tensor_scalar(
        out=out, in0=a[:], scalar1=0.0, scalar2=None,
        op0=mybir.AluOpType.max,
    )


def _build(loop_n=0):
    _patch_tile_drain()
    nc = bacc.Bacc()

    xin = nc.dram_tensor("xin", [C, HW], F32, kind="ExternalInput")
    offw = nc.dram_tensor("offw", [4, 128, NCHUNK * KCOL], F32,
                          kind="ExternalInput")
    imaps = nc.dram_tensor("imaps", [2, 128, NCHUNK * KCOL], F32,
                           kind="ExternalInput")
    offyx = nc.dram_tensor("offyx", [100, HW], F32, kind="ExternalInput")
    baseyx = nc.dram_tensor("baseyx", [100, HW], F32, kind="ExternalInput")
    maskn = nc.dram_tensor("maskn", [G * K, HW], F32, kind="ExternalInput")
    sel2 = nc.dram_tensor("sel2", [2, 128], F32, kind="ExternalInput")
    y = nc.dram_tensor("y", [C, HW], F32, kind="ExternalOutput")

    import contextlib

    with tile.TileContext(nc) as tc:
        nc.gpsimd.load_library(library_config.ap_gather)
        with tc.tile_pool(name="main", bufs=1) as P:
            loop_cm = tc.For_i(0, loop_n, 1) if loop_n else \
                contextlib.nullcontext()
            with loop_cm:
                sel2_t = P.tile([2, 128], F32, tag="sel2")
                nc.sync.dma_start(sel2_t[:], sel2[:])

                for p in range(2):
                    with tc.tile_pool(name=f"gp{p}", bufs=1) as GP:
                        # interleaved patch image X4[c, 4q+e]
                        x4 = GP.tile([128, 4 * HW], F32, tag="x4")
                        with tc.tile_pool(name=f"xl{p}", bufs=1) as XL:
                            xt = XL.tile([128, HW], F32, tag="x")
                            nc.sync.dma_start(
                                xt[:], xin[128 * p:128 * (p + 1), :])
                            for e, s in enumerate((0, 1, 64, 65)):
                                n = HW - s
                                nc.vector.tensor_copy(
                                    x4[:, e:e + 4 * (n - 1) + 1:4], xt[:, s:HW])

                        for ch in range(NCHUNK):
                            _chunk(nc, tc, GP, p, ch, x4, sel2_t, offw,
                                   imaps, offyx, baseyx, maskn, y)
    nc.finalize()
    return nc


def _chunk(nc, tc, GP, p, ch, x4, sel2_t, offw, imaps, offyx, baseyx,
           maskn, y):
    c0, c1 = ch * CH, (ch + 1) * CH
    k0, k1 = ch * KCOL, (ch + 1) * KCOL
    sh = [128, KCOL]
    with tc.tile_pool(name=f"chunk{p}{ch}", bufs=1) as CP:
        qi = CP.tile(sh, I16, tag="qb")
        w4 = CP.tile([36, 4 * CH], F32, tag="w4")

        # ---------------- idx path: patch base qb ----------------
        with tc.tile_pool(name=f"idxtmp{p}{ch}", bufs=1) as IT:
            py = IT.tile(sh, F32, tag="py")
            px = IT.tile(sh, F32, tag="px")
            im = IT.tile(sh, F32, tag="im")
            jm = IT.tile(sh, F32, tag="jm")
            nc.sync.dma_start(py[:], offw[2 * p, :, k0:k1])
            nc.sync.dma_start(px[:], offw[2 * p + 1, :, k0:k1])
            nc.sync.dma_start(im[:], imaps[0, :, k0:k1])
            nc.sync.dma_start(jm[:], imaps[1, :, k0:k1])
            nc.vector.tensor_tensor(py[:], py[:], im[:],
                                    mybir.AluOpType.add)
            nc.vector.tensor_tensor(px[:], px[:], jm[:],
                                    mybir.AluOpType.add)
            fy = IT.tile(sh, F32, tag="fy")
            fx = IT.tile(sh, F32, tag="fx")
            _floor(nc, IT, fy[:], py[:], sh, "f")
            _floor(nc, IT, fx[:], px[:], sh, "f")
            _clamp(nc, fy[:], fy[:], 0.0, 62.0)
            _clamp(nc, fx[:], fx[:], 0.0, 62.0)
            nc.vector.tensor_scalar_mul(fy[:], fy[:], 64.0)
            nc.vector.tensor_tensor(fy[:], fy[:], fx[:],
                                    mybir.AluOpType.add)
            nc.any.tensor_copy(qi[:], fy[:])

        # ------------- weight path: tents at patch positions -------------
        with tc.tile_pool(name=f"wtmp{p}{ch}", bufs=1) as WT:
            shw = [100, CH]
            pyx = WT.tile(shw, F32, tag="pyx")
            bas = WT.tile(shw, F32, tag="bas")
            nc.sync.dma_start(pyx[:], offyx[:, c0:c1])
            nc.sync.dma_start(bas[:], baseyx[:, c0:c1])
            nc.vector.tensor_tensor(pyx[:], pyx[:], bas[:],
                                    mybir.AluOpType.add)
            f = WT.tile(shw, F32, tag="f")
            _floor(nc, WT, f[:], pyx[:], shw, "wf")
            _clamp(nc, f[:], f[:], 0.0, 62.0)
            u = WT.tile(shw, F32, tag="u")
            nc.vector.tensor_tensor(u[:], pyx[:], f[:],
                                    mybir.AluOpType.subtract)
            t0 = WT.tile(shw, F32, tag="t0")
            t1 = WT.tile(shw, F32, tag="t1")
            _tent(nc, WT, t0[:], u[:], shw, "T0")
            nc.vector.tensor_scalar_add(u[:], u[:], -1.0)
            _tent(nc, WT, t1[:], u[:], shw, "T1")
            tx0 = WT.tile([36, CH], F32, tag="tx0")
            tx1 = WT.tile([36, CH], F32, tag="tx1")
            nc.any.tensor_copy(tx0[:], t0[64:100, :])
            nc.any.tensor_copy(tx1[:], t1[64:100, :])
            msk = WT.tile([36, CH], F32, tag="msk")
            nc.sync.dma_start(msk[:], maskn[:, c0:c1])
            ty = {0: t0, 1: t1}
            tx = {0: tx0, 1: tx1}
            wtmp = WT.tile([36, CH], F32, tag="wprod")
            for (ey, ex) in E4:
                e = 2 * ey + ex
                nc.vector.tensor_tensor(
                    wtmp[:], ty[ey][0:36, :], tx[ex][:],
                    mybir.AluOpType.mult)
                nc.vector.tensor_tensor(
                    w4[:, e:e + 4 * (CH - 1) + 1:4], wtmp[:], msk[:],
                    mybir.AluOpType.mult)

        # -------- gathers + replication + combine --------
        with tc.tile_pool(name=f"work{p}{ch}", bufs=2) as WK, \
             tc.tile_pool(name=f"gp{p}{ch}", bufs=3) as GPO, \
             tc.tile_pool(name=f"prp{p}{ch}", bufs=1) as PRP, \
             tc.tile_pool(name=f"acc{p}{ch}", bufs=1) as AP_, \
             tc.tile_pool(name=f"psum{p}{ch}", bufs=2,
                          space="PSUM") as PS:
            acc = AP_.tile([128, CH], F32, tag="acc")
            for k in range(K):
                idx_ap = qi[:, k * S_CH:(k + 1) * S_CH]
                gt = GPO.tile([128, 4 * CH], F32, tag="g")
                nc.gpsimd.ap_gather(
                    gt[:], x4[:], idx_ap,
                    channels=128, num_elems=HW, d=4, num_idxs=CH)
                # replicate w4 rows {(2p)*9+k, (2p+1)*9+k}
                pr = PRP.tile([2, 4 * CH], F32, tag="pairs")
                r0 = 18 * p + k
                nc.sync.dma_start(pr[0:1, :], w4[r0:r0 + 1, :])
                nc.sync.dma_start(pr[1:2, :], w4[r0 + 9:r0 + 10, :])
                wr = WK.tile([128, 4 * CH], F32, tag="wrep")
                for half in range(2):
                    pt = PS.tile([128, 2 * CH], F32, tag="ps")
                    for jc in range(2 * CH // 512):
                        o0 = half * 2 * CH + jc * 512
                        nc.tensor.matmul(
                            pt[:, jc * 512:(jc + 1) * 512],
                            sel2_t[:], pr[:, o0:o0 + 512],
                            start=True, stop=True)
                    nc.scalar.activation(
                        wr[:, half * 2 * CH:(half + 1) * 2 * CH],
                        pt[:], mybir.ActivationFunctionType.Copy)
                # products in place into gt, then pairwise reduce
                nc.vector.tensor_tensor(gt[:], gt[:], wr[:],
                                        mybir.AluOpType.mult)
                t2 = WK.tile([128, 2 * CH], F32, tag="t2")
                nc.vector.tensor_tensor(
                    t2[:], gt[:, 0:4 * CH:2], gt[:, 1:4 * CH:2],
                    mybir.AluOpType.add)
                if k == 0:
                    nc.vector.tensor_tensor(
                        acc[:], t2[:, 0:2 * CH:2], t2[:, 1:2 * CH:2],
                        mybir.AluOpType.add)
                else:
                    t1_ = WK.tile([128, CH], F32, tag="t1_")
                    nc.vector.tensor_tensor(
                        t1_[:], t2[:, 0:2 * CH:2], t2[:, 1:2 * CH:2],
                        mybir.AluOpType.add)
                    nc.vector.tensor_tensor(
                        acc[:], acc[:], t1_[:], mybir.AluOpType.add)
            nc.sync.dma_start(y[128 * p:128 * (p + 1), c0:c1], acc[:])


def _host_prep(input_b, offset_b, mask_b, consts):
    xin = np.ascontiguousarray(input_b.reshape(C, HW), dtype=np.float32)
    off = offset_b.reshape(G, K, 2, HW)

    off_r = off.reshape(2, 2, K, 2, NCHUNK, S_CH, 16)  # [p,gi,k,c,ch,scol,l]
    t = off_r.transpose(0, 3, 1, 6, 4, 2, 5)           # [p,c,gi,l,ch,k,scol]
    t = np.broadcast_to(t[:, :, :, None], (2, 2, 2, 4, 16, NCHUNK, K, S_CH))
    offw = np.ascontiguousarray(
        t.reshape(4, 128, NCHUNK * KCOL), dtype=np.float32)

    offyx = np.zeros((100, HW), dtype=np.float32)
    offyx[0:36] = off[:, :, 0].reshape(G * K, HW)
    offyx[64:100] = off[:, :, 1].reshape(G * K, HW)

    return {
        "xin": xin,
        "offw": offw,
        "imaps": consts["imaps"],
        "offyx": offyx,
        "baseyx": consts["baseyx"],
        "maskn": np.ascontiguousarray(mask_b.reshape(G * K, HW),
                                      dtype=np.float32),
        "sel2": consts["sel2"],
    }


def _consts():
    imaps = np.empty((2, 128, NCHUNK * KCOL), dtype=np.float32)
    lane = np.arange(128) % 16
    for ch in range(NCHUNK):
        for k in range(K):
            for sc in range(S_CH):
                s = ch * S_CH + sc
                jj = 16 * s + lane
                col = ch * KCOL + k * S_CH + sc
                imaps[0, :, col] = jj // 64 - 1 + KY[k]
                imaps[1, :, col] = jj % 64 - 1 + KX[k]

    baseyx = np.zeros((100, HW), dtype=np.float32)
    j = np.arange(HW)
    for g in range(G):
        for k in range(K):
            baseyx[g * K + k] = j // 64 - 1 + KY[k]
            baseyx[64 + g * K + k] = j % 64 - 1 + KX[k]

    sel2 = np.zeros((2, 128), dtype=np.float32)
    sel2[0, 0:64] = 1.0
    sel2[1, 64:128] = 1.0
    return {"imaps": imaps, "baseyx": baseyx, "sel2": sel2}


_STATE = {}


def kernel(input, offset, mask):
    if "nc" not in _STATE:
        _STATE["nc"] = _build()
        _STATE["consts"] = _consts()
    nc = _STATE["nc"]
    consts = _STATE["consts"]
    in_maps = [
        _host_prep(np.asarray(input[b]), np.asarray(offset[b]),
                   np.asarray(mask[b]), consts)
        for b in range(B)
    ]
    res = run_bass_kernel_spmd(nc, in_maps, core_ids=list(range(B)))
    out = np.stack([res.results[b]["y"].reshape(C, H, W) for b in range(B)])
    return out
